# revision 4
# baseline (speedup 1.0000x reference)
"""Trainium2 Bass kernel for nn_AttLayer (attention pooling).

Reference computation (per sample b):
    uit = tanh(x @ W + b)            # [T, D]
    ait = uit @ u                    # [T]
    a   = exp(ait); a /= (sum(a) + 1e-7)
    out = a @ x                      # [D]

Sharding: data-parallel over batch B=32 across 8 cores (4 samples/core);
W/b/u replicated. No cross-core communication.

Device-side layout: the host feeds x pre-transposed per sample
(xT [D, T], partition dim = d) so the x@W contraction over d maps onto
the PE array with W chunks as the stationary operand — no on-chip
transpose of x. Per t-chunk of 512:
  1. PE: uitT[e, t] accumulated over 4 K-chunks (float32r, full rate)
  2. ACT: tanh(+ per-partition bias b[e]) PSUM -> SBUF (float32r)
  3. PE: ait[1, t] = u-weighted partition reduction (u chunks as weights)
  4. ACT: copy ait PSUM -> SBUF (f32r)
  5. PE: broadcast ait across 128 partitions (ones[1,128] as weights, K=1)
  6. ACT: exp PSUM -> SBUF [128, t] + per-partition chunk sums (accum_out)
  7. DVE: pooling partial[d] += sum_t exp_bcast[*, t] * xT[d, t]
Then one reciprocal of (sum + eps) per sample (available in every
partition thanks to step 6) scales the pooled vector.

Matmuls run in float32r (TF32-like, ~11-bit mantissa, full PE rate at
N=512, same 4-byte storage as fp32); the pooling path reads x as true
fp32 so output precision is set by fp32 x and the attention weights.

NOTE: nc.vector.tensor_tensor_reduce (native DVE TENSOR_TENSOR_REDUCE)
crashes TRN2 hardware (NRT_EXEC_UNIT_UNRECOVERABLE) — bisected on HW.
Pooling uses either affine_mul_reduce (custom DVE ucode) or plain
tensor_tensor + reduce_sum, selected by USE_AMR.
"""

import numpy as np

import concourse.bass as bass  # noqa: F401
import concourse.tile as tile
import concourse.mybir as mybir
from concourse import bacc, bass_utils

f32 = mybir.dt.float32
f32r = mybir.dt.float32r
AF = mybir.ActivationFunctionType
ALU = mybir.AluOpType

B, T, D = 32, 2048, 512
NCORES = 8
SPC = B // NCORES        # samples per core
TCH = 512                # t-chunk (matmul moving dim / PSUM bank width)
NTC = T // TCH           # t-chunks per sample
NDC = D // 128           # contraction chunks (and d-tiles of xT)
NEC = D // 128           # e-tiles of uitT
EPS = 1e-7

USE_AMR = True           # fused multiply+reduce pooling via custom DVE op


def build():
    nc = bacc.Bacc("TRN2", target_bir_lowering=False, debug=False)

    xT = nc.dram_tensor("xT", [SPC, D, T], f32r, kind="ExternalInput").ap()
    W = nc.dram_tensor("W", [D, D], f32r, kind="ExternalInput").ap()
    b = nc.dram_tensor("b", [D], f32, kind="ExternalInput").ap()
    u = nc.dram_tensor("u", [D], f32r, kind="ExternalInput").ap()
    ones = nc.dram_tensor("ones", [1, 128], f32r, kind="ExternalInput").ap()
    # out[s, dt, p] == pooled[b=s, d=dt*128+p]; host reshapes to [SPC, D]
    out = nc.dram_tensor("out", [SPC * NDC, 128], f32, kind="ExternalOutput").ap()

    with tile.TileContext(nc) as tc:
        with (
            tc.tile_pool(name="consts", bufs=1) as cpool,
            tc.tile_pool(name="x", bufs=2) as xpool,
            tc.tile_pool(name="th", bufs=4) as thpool,
            tc.tile_pool(name="a", bufs=2) as apool,
            tc.tile_pool(name="s", bufs=2) as spool,
            tc.tile_pool(name="scr", bufs=3) as scrpool,
            tc.tile_pool(name="po", bufs=2) as popool,
            tc.tile_pool(name="psU", bufs=3, space="PSUM") as psU,
            tc.tile_pool(name="psA", bufs=2, space="PSUM") as psA,
            tc.tile_pool(name="psB", bufs=2, space="PSUM") as psB,
        ):
            # ---- constants (loaded once) ----
            w_sb = cpool.tile([128, NDC * D], f32r)  # [128d, (dc, e)]
            for dc in range(NDC):
                nc.sync.dma_start(w_sb[:, dc * D:(dc + 1) * D],
                                  W[dc * 128:(dc + 1) * 128, :])
            b_sb = cpool.tile([128, NEC], f32)
            nc.sync.dma_start(b_sb[:], b.rearrange("(c p) -> p c", p=128))
            u_sb = cpool.tile([128, NEC], f32r)
            nc.sync.dma_start(u_sb[:], u.rearrange("(c p) -> p c", p=128))
            ones_sb = cpool.tile([1, 128], f32r)
            nc.sync.dma_start(ones_sb[:], ones[:])

            for s in range(SPC):
                # ---- load xT for this sample: 4 d-tiles of [128, T] ----
                xts = []
                for dc in range(NDC):
                    xt = xpool.tile([128, T], f32r, tag=f"x{dc}")
                    nc.sync.dma_start(xt[:], xT[s, dc * 128:(dc + 1) * 128, :])
                    xts.append(xt)

                chunksum = spool.tile([128, NTC], f32, tag="cs")
                pparts = popool.tile([128, NTC * NDC], f32, tag="pp")

                for tci in range(NTC):
                    tsl = slice(tci * TCH, (tci + 1) * TCH)
                    ait_ps = psA.tile([1, TCH], f32)
                    for ec in range(NEC):
                        ps = psU.tile([128, TCH], f32)
                        for dc in range(NDC):
                            nc.tensor.matmul(
                                ps[:],
                                w_sb[:, dc * D + ec * 128: dc * D + (ec + 1) * 128],
                                xts[dc][:, tsl],
                                start=(dc == 0), stop=(dc == NDC - 1),
                            )
                        th = thpool.tile([128, TCH], f32r)
                        nc.scalar.activation(th[:], ps[:], AF.Tanh,
                                             bias=b_sb[:, ec:ec + 1])
                        nc.tensor.matmul(
                            ait_ps[:], u_sb[:, ec:ec + 1], th[:],
                            start=(ec == 0), stop=(ec == NEC - 1),
                        )
                    # ait -> SBUF (f32r) so it can feed the broadcast matmul
                    aitc = apool.tile([1, TCH], f32r, tag="aitc")
                    nc.scalar.activation(aitc[:], ait_ps[:], AF.Copy)
                    # broadcast ait across 128 partitions
                    ab_ps = psB.tile([128, TCH], f32)
                    nc.tensor.matmul(ab_ps[:], ones_sb[:], aitc[:],
                                     start=True, stop=True)
                    # exp (+ per-partition running chunk sum)
                    a_b = apool.tile([128, TCH], f32, tag="ab")
                    nc.scalar.activation(a_b[:], ab_ps[:], AF.Exp,
                                         accum_out=chunksum[:, tci:tci + 1])
                    # pooling partials over this t-chunk
                    for dt in range(NDC):
                        pslot = pparts[:, tci * NDC + dt: tci * NDC + dt + 1]
                        scr = scrpool.tile([128, TCH], f32, tag="scr")
                        if USE_AMR:
                            nc.vector.affine_mul_reduce(
                                out=scr[:], accum_out=pslot,
                                in0=xts[dt][:, tsl].bitcast(f32), in1=a_b[:],
                                scale=1.0, bias=0.0)
                        else:
                            nc.vector.tensor_tensor(
                                out=scr[:], in0=xts[dt][:, tsl].bitcast(f32),
                                in1=a_b[:], op=ALU.mult)
                            nc.vector.reduce_sum(pslot, scr[:],
                                                 axis=mybir.AxisListType.X)

                # ---- normalize and emit ----
                S128 = spool.tile([128, 1], f32, tag="S128")
                nc.vector.reduce_sum(S128[:], chunksum[:],
                                     axis=mybir.AxisListType.X)
                S128e = spool.tile([128, 1], f32, tag="S128e")
                nc.vector.tensor_scalar_add(S128e[:], S128[:], EPS)
                inv128 = spool.tile([128, 1], f32, tag="inv128")
                nc.vector.reciprocal(inv128[:], S128e[:])
                pooled = popool.tile([128, NDC], f32, tag="pooled")
                nc.vector.reduce_sum(
                    pooled[:],
                    pparts[:].rearrange("p (tc dt) -> p dt tc", dt=NDC),
                    axis=mybir.AxisListType.X,
                )
                pooledn = popool.tile([128, NDC], f32, tag="pooledn")
                nc.vector.tensor_scalar_mul(pooledn[:], pooled[:], inv128[:])
                nc.sync.dma_start(
                    out[s * NDC:(s + 1) * NDC, :].transpose([1, 0]), pooledn[:]
                )
    nc.compile()
    return nc


_NC_CACHE = None


def kernel(x: np.ndarray, W: np.ndarray, b: np.ndarray, u: np.ndarray) -> np.ndarray:
    global _NC_CACHE
    assert x.shape == (B, T, D) and W.shape == (D, D)
    x = np.ascontiguousarray(x, dtype=np.float32)
    # host-side pre-transpose: [B, T, D] -> [B, D, T]
    xt = np.ascontiguousarray(np.transpose(x, (0, 2, 1)))
    W = np.ascontiguousarray(W, dtype=np.float32)
    b = np.ascontiguousarray(b, dtype=np.float32)
    u = np.ascontiguousarray(u, dtype=np.float32)
    ones = np.ones((1, 128), dtype=np.float32)

    if _NC_CACHE is None:
        _NC_CACHE = build()
    nc = _NC_CACHE

    in_maps = []
    for c in range(NCORES):
        shard = xt[c * SPC:(c + 1) * SPC]
        in_maps.append({"xT": shard, "W": W, "b": b, "u": u, "ones": ones})

    res = bass_utils.run_bass_kernel_spmd(
        nc, in_maps, core_ids=list(range(NCORES))
    )
    outs = [r["out"].reshape(SPC, D) for r in res.results]
    return np.concatenate(outs, axis=0).astype(np.float32)


if __name__ == "__main__":
    rng = np.random.default_rng(0)
    x = rng.standard_normal((B, T, D)).astype(np.float32)
    W = (rng.standard_normal((D, D)) / np.sqrt(D)).astype(np.float32)
    b = np.zeros(D, np.float32)
    u = (rng.standard_normal(D) / np.sqrt(D)).astype(np.float32)
    out = kernel(x=x, W=W, b=b, u=u)
    print("out", out.shape, out.dtype, float(np.abs(out).max()))
